# revision 22
# baseline (speedup 1.0000x reference)
"""Trainium2 Bass kernel for CrossFrameAbsoluteAttn.

Math (per batch b, head h, with d=32, HW=4096):
    Q = Wq x2 + bq ; K = Wk x1 + bk ; V = Wv up(feat1) + bv
    sim = (Q^T K)/sqrt(d) ; t = Q^T wt + bt
    attn = relu((sim - t)/3000)             # no row normalization
    out = attn V^T ; out_feat = Wp out + bp
    ofd = avgpool2x2(out_feat) ; final = feat2 + ofd

Key folds used here:
  * sim - t = q'.k' with q' = [q; 1], k' = [k/sqrt(d) - wt; -bt]  (K=33 contraction)
  * 1/3000 and the 1/4 of the average pool are folded into V
  * avgpool2x2 commutes with the linear ops after the relu, so attn is pooled
    4:1 right after the relu and everything downstream runs at 32x32 res.
    Queries are ordered (dy, dx, y', x') so the pool is two contiguous
    half-tile adds: DVE fp16 TT (2x mode) then GPSIMD fp16 TT.
  * the tiny 1x1 convs / upsample / weight folds are host-side numpy; the
    device does the quadratic part: scores matmul -> relu -> pool -> PV ->
    per-head projection accumulate.

Sharding: 8 cores split the 8192 (b, query-row) space: core c owns batch c//4,
query rows 16*(c%4) .. 16*(c%4)+16 (1024 queries), all 9 heads. No collectives.
Queries are ordered so each 2x2 pool group is contiguous (4:1 reduce on DVE).
"""

import math
import sys

import numpy as np

sys.path.insert(0, "/opt/trn_rl_repo")

import concourse.bass as bass  # noqa: E402
import concourse.tile as tile  # noqa: E402
from concourse import bacc, mybir  # noqa: E402
from concourse import bass_utils  # noqa: E402

HEADS = 9
EMBED = 32
TEMPERATURE = 3000.0
B, H, W = 2, 64, 64
HW = H * W
N_CORES = 8
NQ = 1024           # queries per core
NG = NQ // 4        # pooled outputs per core
MT = HW // 128      # 32 m-tiles

F16 = mybir.dt.float16
F32 = mybir.dt.float32

_CACHE = {}


def _build(repeat=1):
    if ("nc", repeat) in _CACHE:
        return _CACHE[("nc", repeat)]

    nc = bacc.Bacc("TRN2", target_bir_lowering=False, debug=False,
                   num_devices=N_CORES)

    ka = nc.dram_tensor("ka", [33, HEADS * HW], F16, kind="ExternalInput").ap()
    qa = nc.dram_tensor("qa", [33, HEADS * NQ], F16, kind="ExternalInput").ap()
    vt = nc.dram_tensor("vt", [128, HEADS * MT * EMBED], F16,
                        kind="ExternalInput").ap()
    outh = nc.dram_tensor("outh", [HEADS * 32, NG], F32,
                          kind="ExternalOutput").ap()

    with tile.TileContext(nc) as tc:
        with (
            tc.tile_pool(name="const", bufs=1) as cpool,
            tc.tile_pool(name="attn", bufs=6) as apool,
            tc.tile_pool(name="poolx", bufs=6) as pxpool,
            tc.tile_pool(name="pooled", bufs=2 * MT + 2) as plpool,
            tc.tile_pool(name="outh", bufs=2) as opool,
            tc.tile_pool(name="ps_s", bufs=3, space="PSUM") as ps_s,
            tc.tile_pool(name="ps_pv", bufs=2, space="PSUM") as ps_pv,
        ):
            ka_s = cpool.tile([33, HEADS * HW], F16)
            qa_s = cpool.tile([33, HEADS * NQ], F16)
            vt_s = cpool.tile([128, HEADS * MT * EMBED], F16)
            # per-head, split across queues, in consumption order
            for h in range(HEADS):
                for part in range(4):
                    lo = h * HW + part * (HW // 4)
                    hi = lo + HW // 4
                    nc.sync.dma_start(ka_s[:, lo:hi], ka[:, lo:hi])
                nc.sync.dma_start(qa_s[:, h * NQ:(h + 1) * NQ],
                                  qa[:, h * NQ:(h + 1) * NQ])
                nc.sync.dma_start(
                    vt_s[:, h * MT * EMBED:(h + 1) * MT * EMBED],
                    vt[:, h * MT * EMBED:(h + 1) * MT * EMBED])

            pooled_tiles = {}

            def emit_scores(h):
                for mt in range(MT):
                    ps = ps_s.tile([128, NQ], F32)
                    lhsT = ka_s[:, h * HW + mt * 128: h * HW + (mt + 1) * 128]
                    nc.tensor.matmul(ps[:, 0:512], lhsT,
                                     qa_s[:, h * NQ: h * NQ + 512],
                                     start=True, stop=True)
                    nc.tensor.matmul(ps[:, 512:1024], lhsT,
                                     qa_s[:, h * NQ + 512: h * NQ + 1024],
                                     start=True, stop=True)
                    at = apool.tile([128, NQ], F16)
                    # relu engine split: 11/16 ACT, 5/16 DVE (DVE also pools),
                    # Bresenham-interleaved so neither engine starves
                    if ((h * MT + mt) * 11) % 16 < 11:
                        nc.scalar.activation(at[:], ps[:],
                                             mybir.ActivationFunctionType.Relu)
                    else:
                        nc.vector.tensor_scalar_max(at[:], ps[:], 0.0)
                    px = pxpool.tile([128, NQ // 2], F16, tag="px")
                    nc.vector.tensor_add(px[:], at[:, 0:NQ // 2],
                                         at[:, NQ // 2:NQ])
                    pl = plpool.tile([128, NG], F16, tag="pooled")
                    nc.gpsimd.tensor_add(pl[:], px[:, 0:NG], px[:, NG:2 * NG])
                    pooled_tiles[(h, mt)] = pl

            pv_psum = {}

            def emit_pv_mms(h):
                po = ps_pv.tile([32, NG], F32, tag="po")
                for mt in range(MT):
                    pl = pooled_tiles.pop((h, mt))
                    nc.tensor.matmul(
                        po[:],
                        vt_s[:, h * MT * EMBED + mt * EMBED:
                             h * MT * EMBED + (mt + 1) * EMBED],
                        pl[:], start=(mt == 0), stop=(mt == MT - 1),
                        skip_group_check=True)
                pv_psum[h] = po

            def emit_out(h):
                po = pv_psum.pop(h)
                oh = opool.tile([32, NG], F32, tag="oh")
                nc.vector.tensor_copy(oh[:], po[:])
                nc.sync.dma_start(outh[h * 32:(h + 1) * 32, :], oh[:])

            for _rep in range(repeat):
                emit_scores(0)
                for h in range(1, HEADS):
                    emit_pv_mms(h - 1)
                    emit_scores(h)
                    emit_out(h - 1)
                emit_pv_mms(HEADS - 1)
                emit_out(HEADS - 1)

    nc.compile()
    _CACHE[("nc", repeat)] = nc
    return nc


def _upsample2x(x):
    """[C,32,32] -> [C,64,64] bilinear, half-pixel centers, edge clamp."""
    C, h, w = x.shape
    idx = np.arange(2 * h)
    pos = 0.5 * idx - 0.25
    lo = np.floor(pos).astype(int)
    frac = (pos - lo).astype(np.float32)
    lo_c = np.clip(lo, 0, h - 1)
    hi_c = np.clip(lo + 1, 0, h - 1)
    y = x[:, lo_c, :] * (1 - frac)[None, :, None] + x[:, hi_c, :] * frac[None, :, None]
    z = y[:, :, lo_c] * (1 - frac)[None, None, :] + y[:, :, hi_c] * frac[None, None, :]
    return z.astype(np.float32)


def kernel(second_frame, first_frame_aligned, second_frame_feat,
           first_frame_feat_aligned, Wq, bq, Wk, bk, Wv, bv, Wp, bp, Wt, bt):
    second_frame = np.asarray(second_frame, np.float32)
    first_frame_aligned = np.asarray(first_frame_aligned, np.float32)
    second_frame_feat = np.asarray(second_frame_feat, np.float32)
    first_frame_feat_aligned = np.asarray(first_frame_feat_aligned, np.float32)
    Wq = np.asarray(Wq, np.float32); bq = np.asarray(bq, np.float32)
    Wk = np.asarray(Wk, np.float32); bk = np.asarray(bk, np.float32)
    Wv = np.asarray(Wv, np.float32); bv = np.asarray(bv, np.float32)
    Wp = np.asarray(Wp, np.float32); bp = np.asarray(bp, np.float32)
    Wt = np.asarray(Wt, np.float32); bt = np.asarray(bt, np.float32)

    nc = _build()

    s = 1.0 / math.sqrt(EMBED)
    wt = Wt[0]                                   # [32], shared across heads

    # per-batch host prep ----------------------------------------------------
    ka_b, qfull_b, vt_b = [], [], []
    for b in range(B):
        f1 = first_frame_aligned[b].reshape(3, HW)
        f2 = second_frame[b].reshape(3, HW)
        fu = _upsample2x(first_frame_feat_aligned[b]).reshape(4, HW)

        # K' per head: [33, HW]; rows k/sqrt(d)-wt, last row -bt
        kfull = (Wk * s) @ f1 + (bk * s)[:, None]          # [288, HW]
        kfull = kfull.reshape(HEADS, EMBED, HW) - wt[None, :, None]
        ka = np.empty((33, HEADS * HW), np.float16)
        for h in range(HEADS):
            ka[:EMBED, h * HW:(h + 1) * HW] = kfull[h]
            ka[EMBED, h * HW:(h + 1) * HW] = -bt[0]
        ka_b.append(ka)

        qfull_b.append((Wq @ f2 + bq[:, None]).reshape(HEADS, EMBED, H, W))

        # V'^T: [HW, 288] scaled; -> [p, h, mt, d]
        vtf = (fu.T @ Wv.T + bv[None, :]) / (TEMPERATURE * 4.0)   # [HW, 288]
        vtr = vtf.reshape(MT, 128, HEADS, EMBED)
        vt_b.append(np.ascontiguousarray(
            vtr.transpose(1, 2, 0, 3).reshape(128, HEADS * MT * EMBED)
        ).astype(np.float16))

    in_maps = []
    for c in range(N_CORES):
        b, y0 = c // 4, (c % 4) * 16
        # grouped query order: (dy, dx, y', x') -> pool = two half adds
        qc = qfull_b[b][:, :, y0:y0 + 16, :]               # [9, 32, 16, 64]
        qg = qc.reshape(HEADS, EMBED, 8, 2, 32, 2)         # y',dy,x',dx
        qg = qg.transpose(0, 1, 3, 5, 2, 4).reshape(HEADS, EMBED, NQ)
        qa = np.empty((33, HEADS * NQ), np.float16)
        for h in range(HEADS):
            qa[:EMBED, h * NQ:(h + 1) * NQ] = qg[h]
            qa[EMBED, h * NQ:(h + 1) * NQ] = 1.0
        in_maps.append({"ka": ka_b[b], "qa": qa, "vt": vt_b[b]})

    res = bass_utils.run_bass_kernel_spmd(nc, in_maps,
                                          core_ids=list(range(N_CORES)))

    out_ofd = np.zeros((B, 4, 32, 32), np.float32)
    for c in range(N_CORES):
        b, y0 = c // 4, (c % 4) * 16
        oh = res.results[c]["outh"]                      # [288, 256]
        ofd = Wp @ oh + bp[:, None]                      # [4, 256]
        out_ofd[b, :, y0 // 2: y0 // 2 + 8, :] = ofd.reshape(4, 8, 32)
    out_fin = second_frame_feat + out_ofd
    return out_fin, out_ofd


# revision 25
# speedup vs baseline: 1.0096x; 1.0096x over previous
"""Trainium2 Bass kernel for CrossFrameAbsoluteAttn.

Math (per batch b, head h, with d=32, HW=4096):
    Q = Wq x2 + bq ; K = Wk x1 + bk ; V = Wv up(feat1) + bv
    sim = (Q^T K)/sqrt(d) ; t = Q^T wt + bt
    attn = relu((sim - t)/3000)             # no row normalization
    out = attn V^T ; out_feat = Wp out + bp
    ofd = avgpool2x2(out_feat) ; final = feat2 + ofd

Key folds used here:
  * sim - t = q'.k' with q' = [q; 1], k' = [k/sqrt(d) - wt; -bt]  (K=33 contraction)
  * 1/3000 and the 1/4 of the average pool are folded into V
  * avgpool2x2 commutes with the linear ops after the relu, so attn is pooled
    4:1 right after the relu and everything downstream runs at 32x32 res.
    Queries are ordered (dy, dx, y', x') so the pool is two contiguous
    half-tile adds: DVE fp16 TT (2x mode) then GPSIMD fp16 TT.
  * the tiny 1x1 convs / upsample / weight folds are host-side numpy; the
    device does the quadratic part: scores matmul -> relu -> pool -> PV ->
    per-head projection accumulate.

Sharding: 8 cores split the 8192 (b, query-row) space: core c owns batch c//4,
query rows 16*(c%4) .. 16*(c%4)+16 (1024 queries), all 9 heads. No collectives.
Queries are ordered so each 2x2 pool group is contiguous (4:1 reduce on DVE).
"""

import math
import sys

import numpy as np

sys.path.insert(0, "/opt/trn_rl_repo")

import concourse.bass as bass  # noqa: E402
import concourse.tile as tile  # noqa: E402
from concourse import bacc, mybir  # noqa: E402
from concourse import bass_utils  # noqa: E402

HEADS = 9
EMBED = 32
TEMPERATURE = 3000.0
B, H, W = 2, 64, 64
HW = H * W
N_CORES = 8
NQ = 1024           # queries per core
NG = NQ // 4        # pooled outputs per core
MT = HW // 128      # 32 m-tiles

F16 = mybir.dt.float16
F32 = mybir.dt.float32

_CACHE = {}


def _build(repeat=1):
    if ("nc", repeat) in _CACHE:
        return _CACHE[("nc", repeat)]

    nc = bacc.Bacc("TRN2", target_bir_lowering=False, debug=False,
                   num_devices=N_CORES)

    ka = nc.dram_tensor("ka", [33, HEADS * HW], F16, kind="ExternalInput").ap()
    qa = nc.dram_tensor("qa", [33, HEADS * NQ], F16, kind="ExternalInput").ap()
    vt = nc.dram_tensor("vt", [128, HEADS * MT * EMBED], F16,
                        kind="ExternalInput").ap()
    outh = nc.dram_tensor("outh", [HEADS * 32, NG], F16,
                          kind="ExternalOutput").ap()

    with tile.TileContext(nc) as tc:
        with (
            tc.tile_pool(name="const", bufs=1) as cpool,
            tc.tile_pool(name="attn", bufs=6) as apool,
            tc.tile_pool(name="poolx", bufs=6) as pxpool,
            tc.tile_pool(name="pooled", bufs=2 * MT + 2) as plpool,
            tc.tile_pool(name="outh", bufs=2) as opool,
            tc.tile_pool(name="ps_s", bufs=3, space="PSUM") as ps_s,
            tc.tile_pool(name="ps_pv", bufs=2, space="PSUM") as ps_pv,
        ):
            ka_s = cpool.tile([33, HEADS * HW], F16)
            qa_s = cpool.tile([33, HEADS * NQ], F16)
            vt_s = cpool.tile([128, HEADS * MT * EMBED], F16)
            # per-head, split across queues, in consumption order
            for h in range(HEADS):
                for part in range(4):
                    lo = h * HW + part * (HW // 4)
                    hi = lo + HW // 4
                    nc.sync.dma_start(ka_s[:, lo:hi], ka[:, lo:hi])
                nc.sync.dma_start(qa_s[:, h * NQ:(h + 1) * NQ],
                                  qa[:, h * NQ:(h + 1) * NQ])
                nc.sync.dma_start(
                    vt_s[:, h * MT * EMBED:(h + 1) * MT * EMBED],
                    vt[:, h * MT * EMBED:(h + 1) * MT * EMBED])

            pooled_tiles = {}

            def emit_scores(h):
                for mt in range(MT):
                    ps = ps_s.tile([128, NQ], F32)
                    lhsT = ka_s[:, h * HW + mt * 128: h * HW + (mt + 1) * 128]
                    nc.tensor.matmul(ps[:, 0:512], lhsT,
                                     qa_s[:, h * NQ: h * NQ + 512],
                                     start=True, stop=True)
                    nc.tensor.matmul(ps[:, 512:1024], lhsT,
                                     qa_s[:, h * NQ + 512: h * NQ + 1024],
                                     start=True, stop=True)
                    at = apool.tile([128, NQ], F16)
                    # relu engine split: 11/16 ACT, 5/16 DVE (DVE also pools),
                    # Bresenham-interleaved so neither engine starves
                    if ((h * MT + mt) * 11) % 16 < 11:
                        nc.scalar.activation(at[:], ps[:],
                                             mybir.ActivationFunctionType.Relu)
                    else:
                        nc.vector.tensor_scalar_max(at[:], ps[:], 0.0)
                    px = pxpool.tile([128, NQ // 2], F16, tag="px")
                    nc.vector.tensor_add(px[:], at[:, 0:NQ // 2],
                                         at[:, NQ // 2:NQ])
                    pl = plpool.tile([128, NG], F16, tag="pooled")
                    nc.gpsimd.tensor_add(pl[:], px[:, 0:NG], px[:, NG:2 * NG])
                    pooled_tiles[(h, mt)] = pl

            pv_psum = {}

            def emit_pv_mms(h):
                po = ps_pv.tile([32, NG], F32, tag="po")
                for mt in range(MT):
                    pl = pooled_tiles.pop((h, mt))
                    nc.tensor.matmul(
                        po[:],
                        vt_s[:, h * MT * EMBED + mt * EMBED:
                             h * MT * EMBED + (mt + 1) * EMBED],
                        pl[:], start=(mt == 0), stop=(mt == MT - 1),
                        skip_group_check=True)
                pv_psum[h] = po

            def emit_out(h):
                po = pv_psum.pop(h)
                oh = opool.tile([32, NG], F16, tag="oh")
                nc.vector.tensor_copy(oh[:], po[:])
                nc.sync.dma_start(outh[h * 32:(h + 1) * 32, :], oh[:])

            for _rep in range(repeat):
                emit_scores(0)
                for h in range(1, HEADS):
                    emit_pv_mms(h - 1)
                    emit_scores(h)
                    emit_out(h - 1)
                emit_pv_mms(HEADS - 1)
                emit_out(HEADS - 1)

    nc.compile()
    _CACHE[("nc", repeat)] = nc
    return nc


def _upsample2x(x):
    """[C,32,32] -> [C,64,64] bilinear, half-pixel centers, edge clamp."""
    C, h, w = x.shape
    idx = np.arange(2 * h)
    pos = 0.5 * idx - 0.25
    lo = np.floor(pos).astype(int)
    frac = (pos - lo).astype(np.float32)
    lo_c = np.clip(lo, 0, h - 1)
    hi_c = np.clip(lo + 1, 0, h - 1)
    y = x[:, lo_c, :] * (1 - frac)[None, :, None] + x[:, hi_c, :] * frac[None, :, None]
    z = y[:, :, lo_c] * (1 - frac)[None, None, :] + y[:, :, hi_c] * frac[None, None, :]
    return z.astype(np.float32)


def kernel(second_frame, first_frame_aligned, second_frame_feat,
           first_frame_feat_aligned, Wq, bq, Wk, bk, Wv, bv, Wp, bp, Wt, bt):
    second_frame = np.asarray(second_frame, np.float32)
    first_frame_aligned = np.asarray(first_frame_aligned, np.float32)
    second_frame_feat = np.asarray(second_frame_feat, np.float32)
    first_frame_feat_aligned = np.asarray(first_frame_feat_aligned, np.float32)
    Wq = np.asarray(Wq, np.float32); bq = np.asarray(bq, np.float32)
    Wk = np.asarray(Wk, np.float32); bk = np.asarray(bk, np.float32)
    Wv = np.asarray(Wv, np.float32); bv = np.asarray(bv, np.float32)
    Wp = np.asarray(Wp, np.float32); bp = np.asarray(bp, np.float32)
    Wt = np.asarray(Wt, np.float32); bt = np.asarray(bt, np.float32)

    nc = _build()

    s = 1.0 / math.sqrt(EMBED)
    wt = Wt[0]                                   # [32], shared across heads

    # per-batch host prep ----------------------------------------------------
    ka_b, qfull_b, vt_b = [], [], []
    for b in range(B):
        f1 = first_frame_aligned[b].reshape(3, HW)
        f2 = second_frame[b].reshape(3, HW)
        fu = _upsample2x(first_frame_feat_aligned[b]).reshape(4, HW)

        # K' per head: [33, HW]; rows k/sqrt(d)-wt, last row -bt
        kfull = (Wk * s) @ f1 + (bk * s)[:, None]          # [288, HW]
        kfull = kfull.reshape(HEADS, EMBED, HW) - wt[None, :, None]
        ka = np.empty((33, HEADS * HW), np.float16)
        for h in range(HEADS):
            ka[:EMBED, h * HW:(h + 1) * HW] = kfull[h]
            ka[EMBED, h * HW:(h + 1) * HW] = -bt[0]
        ka_b.append(ka)

        qfull_b.append((Wq @ f2 + bq[:, None]).reshape(HEADS, EMBED, H, W))

        # V'^T: [HW, 288] scaled; -> [p, h, mt, d]
        vtf = (fu.T @ Wv.T + bv[None, :]) / (TEMPERATURE * 4.0)   # [HW, 288]
        vtr = vtf.reshape(MT, 128, HEADS, EMBED)
        vt_b.append(np.ascontiguousarray(
            vtr.transpose(1, 2, 0, 3).reshape(128, HEADS * MT * EMBED)
        ).astype(np.float16))

    in_maps = []
    for c in range(N_CORES):
        b, y0 = c // 4, (c % 4) * 16
        # grouped query order: (dy, dx, y', x') -> pool = two half adds
        qc = qfull_b[b][:, :, y0:y0 + 16, :]               # [9, 32, 16, 64]
        qg = qc.reshape(HEADS, EMBED, 8, 2, 32, 2)         # y',dy,x',dx
        qg = qg.transpose(0, 1, 3, 5, 2, 4).reshape(HEADS, EMBED, NQ)
        qa = np.empty((33, HEADS * NQ), np.float16)
        for h in range(HEADS):
            qa[:EMBED, h * NQ:(h + 1) * NQ] = qg[h]
            qa[EMBED, h * NQ:(h + 1) * NQ] = 1.0
        in_maps.append({"ka": ka_b[b], "qa": qa, "vt": vt_b[b]})

    res = bass_utils.run_bass_kernel_spmd(nc, in_maps,
                                          core_ids=list(range(N_CORES)))

    out_ofd = np.zeros((B, 4, 32, 32), np.float32)
    for c in range(N_CORES):
        b, y0 = c // 4, (c % 4) * 16
        oh = res.results[c]["outh"].astype(np.float32)   # [288, 256]
        ofd = Wp @ oh + bp[:, None]                      # [4, 256]
        out_ofd[b, :, y0 // 2: y0 // 2 + 8, :] = ofd.reshape(4, 8, 32)
    out_fin = second_frame_feat + out_ofd
    return out_fin, out_ofd


# revision 29
# speedup vs baseline: 1.0430x; 1.0331x over previous
"""Trainium2 Bass kernel for CrossFrameAbsoluteAttn.

Math (per batch b, head h, with d=32, HW=4096):
    Q = Wq x2 + bq ; K = Wk x1 + bk ; V = Wv up(feat1) + bv
    sim = (Q^T K)/sqrt(d) ; t = Q^T wt + bt
    attn = relu((sim - t)/3000)             # no row normalization
    out = attn V^T ; out_feat = Wp out + bp
    ofd = avgpool2x2(out_feat) ; final = feat2 + ofd

Key folds used here:
  * sim - t = q'.k' with q' = [q; 1], k' = [k/sqrt(d) - wt; -bt]  (K=33 contraction)
  * 1/3000 and the 1/4 of the average pool are folded into V
  * avgpool2x2 commutes with the linear ops after the relu, so attn is pooled
    4:1 right after the relu and everything downstream runs at 32x32 res.
    Queries are ordered (dy, dx, y', x') so the pool is two contiguous
    half-tile adds: DVE fp16 TT (2x mode) then GPSIMD fp16 TT.
  * the tiny 1x1 convs / upsample / weight folds / final 4x288 projection are
    host-side numpy; the device does the quadratic part: scores matmul ->
    relu -> pool -> PV, emitting per-head pooled outputs [32, 256].

Sharding: 8 cores split the 8192 (b, query-row) space: core c owns batch c//4,
query rows 16*(c%4) .. 16*(c%4)+16 (1024 queries), all 9 heads. No collectives.
"""

import math
import sys

import numpy as np

sys.path.insert(0, "/opt/trn_rl_repo")

import concourse.bass as bass  # noqa: E402
import concourse.tile as tile  # noqa: E402
from concourse import bacc, mybir  # noqa: E402
from concourse import bass_utils  # noqa: E402

HEADS = 9
EMBED = 32
TEMPERATURE = 3000.0
B, H, W = 2, 64, 64
HW = H * W
N_CORES = 8
NQ = 1024           # queries per core
NG = NQ // 4        # pooled outputs per core
MT = HW // 128      # 32 m-tiles

F16 = mybir.dt.float16
F32 = mybir.dt.float32

_CACHE = {}


def _build(repeat=1):
    if ("nc", repeat) in _CACHE:
        return _CACHE[("nc", repeat)]

    nc = bacc.Bacc("TRN2", target_bir_lowering=False, debug=False,
                   num_devices=N_CORES)

    ka = nc.dram_tensor("ka", [33, HEADS * HW], F16, kind="ExternalInput").ap()
    qa = nc.dram_tensor("qa", [33, HEADS * NQ], F16, kind="ExternalInput").ap()
    vt = nc.dram_tensor("vt", [128, HEADS * MT * EMBED], F16,
                        kind="ExternalInput").ap()
    outh = nc.dram_tensor("outh", [HEADS * 32, NG], F16,
                          kind="ExternalOutput").ap()

    with tile.TileContext(nc) as tc:
        with (
            tc.tile_pool(name="const", bufs=1) as cpool,
            tc.tile_pool(name="attn", bufs=8) as apool,
            tc.tile_pool(name="poolx", bufs=8) as pxpool,
            tc.tile_pool(name="pooled", bufs=2 * MT + 2) as plpool,
            tc.tile_pool(name="outh", bufs=2) as opool,
            tc.tile_pool(name="ps_s", bufs=3, space="PSUM") as ps_s,
            tc.tile_pool(name="ps_pv", bufs=2, space="PSUM") as ps_pv,
        ):
            ka_s = cpool.tile([33, HEADS * HW], F16)
            qa_s = cpool.tile([33, HEADS * NQ], F16)
            vt_s = cpool.tile([128, HEADS * MT * EMBED], F16)
            # per-head, split across queues, in consumption order
            for h in range(HEADS):
                for part in range(4):
                    lo = h * HW + part * (HW // 4)
                    hi = lo + HW // 4
                    nc.sync.dma_start(ka_s[:, lo:hi], ka[:, lo:hi])
                nc.sync.dma_start(qa_s[:, h * NQ:(h + 1) * NQ],
                                  qa[:, h * NQ:(h + 1) * NQ])
                nc.sync.dma_start(
                    vt_s[:, h * MT * EMBED:(h + 1) * MT * EMBED],
                    vt[:, h * MT * EMBED:(h + 1) * MT * EMBED])

            pooled_tiles = {}

            def emit_scores(h):
                for mt in range(MT):
                    ps = ps_s.tile([128, NQ], F32)
                    lhsT = ka_s[:, h * HW + mt * 128: h * HW + (mt + 1) * 128]
                    nc.tensor.matmul(ps[:, 0:512], lhsT,
                                     qa_s[:, h * NQ: h * NQ + 512],
                                     start=True, stop=True)
                    nc.tensor.matmul(ps[:, 512:1024], lhsT,
                                     qa_s[:, h * NQ + 512: h * NQ + 1024],
                                     start=True, stop=True)
                    at = apool.tile([128, NQ], F16)
                    # relu engine split: 11/16 ACT, 5/16 DVE (DVE also pools),
                    # Bresenham-interleaved so neither engine starves
                    if ((h * MT + mt) * 11) % 16 < 11:
                        nc.scalar.activation(at[:], ps[:],
                                             mybir.ActivationFunctionType.Relu)
                    else:
                        nc.vector.tensor_scalar_max(at[:], ps[:], 0.0)
                    px = pxpool.tile([128, NQ // 2], F16, tag="px")
                    nc.vector.tensor_add(px[:], at[:, 0:NQ // 2],
                                         at[:, NQ // 2:NQ])
                    pl = plpool.tile([128, NG], F16, tag="pooled")
                    nc.gpsimd.tensor_add(pl[:], px[:, 0:NG], px[:, NG:2 * NG])
                    pooled_tiles[(h, mt)] = pl

            pv_psum = {}

            def emit_pv_mms(h):
                po = ps_pv.tile([32, NG], F32, tag="po")
                for mt in range(MT):
                    pl = pooled_tiles.pop((h, mt))
                    nc.tensor.matmul(
                        po[:],
                        vt_s[:, h * MT * EMBED + mt * EMBED:
                             h * MT * EMBED + (mt + 1) * EMBED],
                        pl[:], start=(mt == 0), stop=(mt == MT - 1),
                        skip_group_check=True)
                pv_psum[h] = po

            def emit_out(h):
                po = pv_psum.pop(h)
                oh = opool.tile([32, NG], F16, tag="oh")
                nc.vector.tensor_copy(oh[:], po[:])
                nc.sync.dma_start(outh[h * 32:(h + 1) * 32, :], oh[:])

            for _rep in range(repeat):
                emit_scores(0)
                for h in range(1, HEADS):
                    emit_pv_mms(h - 1)
                    emit_scores(h)
                    emit_out(h - 1)
                emit_pv_mms(HEADS - 1)
                emit_out(HEADS - 1)

    nc.compile()
    _CACHE[("nc", repeat)] = nc
    return nc


def _upsample2x(x):
    """[C,32,32] -> [C,64,64] bilinear, half-pixel centers, edge clamp."""
    C, h, w = x.shape
    idx = np.arange(2 * h)
    pos = 0.5 * idx - 0.25
    lo = np.floor(pos).astype(int)
    frac = (pos - lo).astype(np.float32)
    lo_c = np.clip(lo, 0, h - 1)
    hi_c = np.clip(lo + 1, 0, h - 1)
    y = x[:, lo_c, :] * (1 - frac)[None, :, None] + x[:, hi_c, :] * frac[None, :, None]
    z = y[:, :, lo_c] * (1 - frac)[None, None, :] + y[:, :, hi_c] * frac[None, None, :]
    return z.astype(np.float32)


def kernel(second_frame, first_frame_aligned, second_frame_feat,
           first_frame_feat_aligned, Wq, bq, Wk, bk, Wv, bv, Wp, bp, Wt, bt):
    second_frame = np.asarray(second_frame, np.float32)
    first_frame_aligned = np.asarray(first_frame_aligned, np.float32)
    second_frame_feat = np.asarray(second_frame_feat, np.float32)
    first_frame_feat_aligned = np.asarray(first_frame_feat_aligned, np.float32)
    Wq = np.asarray(Wq, np.float32); bq = np.asarray(bq, np.float32)
    Wk = np.asarray(Wk, np.float32); bk = np.asarray(bk, np.float32)
    Wv = np.asarray(Wv, np.float32); bv = np.asarray(bv, np.float32)
    Wp = np.asarray(Wp, np.float32); bp = np.asarray(bp, np.float32)
    Wt = np.asarray(Wt, np.float32); bt = np.asarray(bt, np.float32)

    nc = _build()

    s = 1.0 / math.sqrt(EMBED)
    wt = Wt[0]                                   # [32], shared across heads

    # per-batch host prep ----------------------------------------------------
    ka_b, qfull_b, vt_b = [], [], []
    for b in range(B):
        f1 = first_frame_aligned[b].reshape(3, HW)
        f2 = second_frame[b].reshape(3, HW)
        fu = _upsample2x(first_frame_feat_aligned[b]).reshape(4, HW)

        # K' per head: [33, HW]; rows k/sqrt(d)-wt, last row -bt
        kfull = (Wk * s) @ f1 + (bk * s)[:, None]          # [288, HW]
        kfull = kfull.reshape(HEADS, EMBED, HW) - wt[None, :, None]
        ka = np.empty((33, HEADS * HW), np.float16)
        for h in range(HEADS):
            ka[:EMBED, h * HW:(h + 1) * HW] = kfull[h]
            ka[EMBED, h * HW:(h + 1) * HW] = -bt[0]
        ka_b.append(ka)

        qfull_b.append((Wq @ f2 + bq[:, None]).reshape(HEADS, EMBED, H, W))

        # V'^T: [HW, 288] scaled; -> [p, h, mt, d]
        vtf = (fu.T @ Wv.T + bv[None, :]) / (TEMPERATURE * 4.0)   # [HW, 288]
        vtr = vtf.reshape(MT, 128, HEADS, EMBED)
        vt_b.append(np.ascontiguousarray(
            vtr.transpose(1, 2, 0, 3).reshape(128, HEADS * MT * EMBED)
        ).astype(np.float16))

    in_maps = []
    for c in range(N_CORES):
        b, y0 = c // 4, (c % 4) * 16
        # grouped query order: (dy, dx, y', x') -> pool = two half adds
        qc = qfull_b[b][:, :, y0:y0 + 16, :]               # [9, 32, 16, 64]
        qg = qc.reshape(HEADS, EMBED, 8, 2, 32, 2)         # y',dy,x',dx
        qg = qg.transpose(0, 1, 3, 5, 2, 4).reshape(HEADS, EMBED, NQ)
        qa = np.empty((33, HEADS * NQ), np.float16)
        for h in range(HEADS):
            qa[:EMBED, h * NQ:(h + 1) * NQ] = qg[h]
            qa[EMBED, h * NQ:(h + 1) * NQ] = 1.0
        in_maps.append({"ka": ka_b[b], "qa": qa, "vt": vt_b[b]})

    res = bass_utils.run_bass_kernel_spmd(nc, in_maps,
                                          core_ids=list(range(N_CORES)))

    out_ofd = np.zeros((B, 4, 32, 32), np.float32)
    for c in range(N_CORES):
        b, y0 = c // 4, (c % 4) * 16
        oh = res.results[c]["outh"].astype(np.float32)   # [288, 256]
        ofd = Wp @ oh + bp[:, None]                      # [4, 256]
        out_ofd[b, :, y0 // 2: y0 // 2 + 8, :] = ofd.reshape(4, 8, 32)
    out_fin = second_frame_feat + out_ofd
    return out_fin, out_ofd
